# revision 1
# baseline (speedup 1.0000x reference)
"""Masked cross-attention kernel for Trainium2 (8 NeuronCores, SPMD).

Problem: B=16 batches of softmax(mask(Q@K^T/sqrt(D)))@V with
Lq=Lk=2048, D=DV=256.  The reference zeroes masked scores (NOT -inf)
before the softmax, so masked keys still contribute exp(0)=1 to the
denominator and weight 1/denom on V rows.

Strategy (all host prep is exact):
  * Zero K rows at k >= valid_length[b] on the host.  Then Q @ K^T is
    *exactly* 0.0 at masked positions — identical to the reference's
    jnp.where — and no mask tensor is needed on-device.
  * Pre-transpose Q and K to [D, L] layout on the host so both matmul
    operands stream naturally (contraction on the partition dim).
  * Append a ones-column to V.  P @ [V | 1] then yields the softmax
    denominator as output column 256 for free.
  * bf16 matmul inputs (fp32 PSUM accumulate), fp32 softmax math.

Per core: 2 batches.  Per batch, for each 512-wide q tile:
  stage 1: S^T[k,q] tiles in PSUM (KtT.T @ Qt), exp via ScalarE
           (scale=1/16 folded in) -> P^T bf16 in SBUF
  stage 2: O[q,v] = (P^T).T @ [V|1] accumulated over k chunks in PSUM;
           divide by column 256 (DVE reciprocal + per-partition mul).
Stage 1 of q-tile i+1 is emitted before stage 2 of q-tile i so the PE
never stalls on the ScalarE exp chain.
"""

import numpy as np
import ml_dtypes

import concourse.bass as bass
import concourse.mybir as mybir
import concourse.tile as tile
from concourse import bacc
from concourse.bass_utils import run_bass_kernel_spmd

B, LQ, LK, D, DV = 16, 2048, 2048, 256, 256
N_CORES = 8
BPC = B // N_CORES  # batches per core

QT = 512            # q-tile width (stage-1 moving free dim)
NQT = LQ // QT      # 4
KT = 128            # k-tile (partition dim of S^T)
NKT = LK // KT      # 16
KG = 2              # k-tiles per PSUM/exp group
NKG = NKT // KG     # 8
NDC = D // 128      # contraction chunks (2)
QS = 128            # q-subtile for stage 2
NQS = QT // QS      # 4
VF = DV + 1         # 257: V plus the ones column

_BF16 = mybir.dt.bfloat16
_F32 = mybir.dt.float32

_NC_CACHE = {}


def _build_nc():
    nc = bacc.Bacc("TRN2", target_bir_lowering=False, debug=False,
                   num_devices=N_CORES)

    qt_d = nc.declare_dram_parameter("qt", [BPC, D, LQ], _BF16, isOutput=False)
    kt_d = nc.declare_dram_parameter("kt", [BPC, D, LK], _BF16, isOutput=False)
    v1_d = nc.declare_dram_parameter("v1", [BPC, LK, VF], _BF16, isOutput=False)
    out_d = nc.declare_dram_parameter("out", [BPC, LQ, DV], _F32, isOutput=True)

    with tile.TileContext(nc) as tc:
        with (
            tc.tile_pool(name="qk", bufs=2) as qk_pool,
            tc.tile_pool(name="v", bufs=2) as v_pool,
            tc.tile_pool(name="p", bufs=2) as p_pool,
            tc.tile_pool(name="osb", bufs=4) as o_pool,
            tc.tile_pool(name="small", bufs=8) as small_pool,
            tc.tile_pool(name="ps_s", bufs=2, space="PSUM") as ps_s,
            tc.tile_pool(name="ps_o", bufs=4, space="PSUM") as ps_o,
        ):
            def load_batch(b):
                kt_sb = qk_pool.tile([128, NDC, LK], _BF16, tag="kt")
                qt_sb = qk_pool.tile([128, NDC, LQ], _BF16, tag="qt")
                v1_sb = v_pool.tile([128, NKT, VF], _BF16, tag="v1")
                nc.sync.dma_start(
                    out=kt_sb, in_=kt_d[b].rearrange("(c p) k -> p c k", p=128))
                nc.sync.dma_start(
                    out=qt_sb, in_=qt_d[b].rearrange("(c p) q -> p c q", p=128))
                nc.sync.dma_start(
                    out=v1_sb, in_=v1_d[b].rearrange("(t p) v -> p t v", p=128))
                return kt_sb, qt_sb, v1_sb

            def stage1(state, qi):
                """S^T = Kt.T @ Qt for one 512-wide q tile; exp -> P^T bf16."""
                kt_sb, qt_sb, _ = state
                p_sb = p_pool.tile([128, NKT * QT], _BF16, tag="p")
                for g in range(NKG):
                    ps = ps_s.tile([128, KG * QT], _F32, tag="s")
                    for h in range(KG):
                        kj = g * KG + h
                        for c in range(NDC):
                            nc.tensor.matmul(
                                ps[:, h * QT:(h + 1) * QT],
                                lhsT=kt_sb[:, c, kj * KT:(kj + 1) * KT],
                                rhs=qt_sb[:, c, qi * QT:(qi + 1) * QT],
                                start=(c == 0), stop=(c == NDC - 1),
                            )
                    nc.scalar.activation(
                        out=p_sb[:, g * KG * QT:(g + 1) * KG * QT], in_=ps,
                        func=mybir.ActivationFunctionType.Exp,
                        scale=1.0 / 16.0)
                return p_sb

            def stage2(state, b, qi, p_sb):
                """O = P @ [V|1]; normalize by the ones column; DMA out."""
                _, _, v1_sb = state
                for s in range(NQS):
                    o_ps = ps_o.tile([128, VF], _F32, tag="o")
                    for kj in range(NKT):
                        nc.tensor.matmul(
                            o_ps,
                            lhsT=p_sb[:, kj * QT + s * QS:kj * QT + (s + 1) * QS],
                            rhs=v1_sb[:, kj, :],
                            start=(kj == 0), stop=(kj == NKT - 1),
                        )
                    recip = small_pool.tile([128, 1], _F32, tag="r")
                    nc.vector.reciprocal(out=recip, in_=o_ps[:, DV:DV + 1])
                    o_sb = o_pool.tile([128, DV], _F32, tag="o_sb")
                    nc.vector.tensor_scalar_mul(
                        out=o_sb, in0=o_ps[:, :DV], scalar1=recip)
                    q0 = qi * QT + s * QS
                    nc.sync.dma_start(out=out_d[b, q0:q0 + QS, :], in_=o_sb)

            states = [load_batch(b) for b in range(BPC)]
            work = [(b, qi) for b in range(BPC) for qi in range(NQT)]
            pending = None  # (state, b, qi, p_sb)
            for b, qi in work:
                p_sb = stage1(states[b], qi)
                if pending is not None:
                    stage2(*pending)
                pending = (states[b], b, qi, p_sb)
            stage2(*pending)

    nc.compile()
    return nc


def _get_nc():
    if "nc" not in _NC_CACHE:
        _NC_CACHE["nc"] = _build_nc()
    return _NC_CACHE["nc"]


def _prepare(query, key, value, valid_length):
    query = np.asarray(query, dtype=np.float32)
    key = np.asarray(key, dtype=np.float32)
    value = np.asarray(value, dtype=np.float32)
    valid_length = np.asarray(valid_length)

    kz = key.copy()
    for b in range(B):
        kz[b, int(valid_length[b]):, :] = 0.0

    bf16 = ml_dtypes.bfloat16
    qt = np.ascontiguousarray(query.transpose(0, 2, 1)).astype(bf16)
    kt = np.ascontiguousarray(kz.transpose(0, 2, 1)).astype(bf16)
    v1 = np.concatenate(
        [value, np.ones((B, LK, 1), np.float32)], axis=-1).astype(bf16)
    return qt, kt, v1


def _run(inputs, trace=False):
    qt, kt, v1 = _prepare(**inputs)
    in_maps = [
        {"qt": qt[c * BPC:(c + 1) * BPC],
         "kt": kt[c * BPC:(c + 1) * BPC],
         "v1": v1[c * BPC:(c + 1) * BPC]}
        for c in range(N_CORES)
    ]
    nc = _get_nc()
    res = run_bass_kernel_spmd(nc, in_maps, core_ids=list(range(N_CORES)),
                               trace=trace)
    out = np.empty((B, LQ, DV), np.float32)
    for c in range(N_CORES):
        out[c * BPC:(c + 1) * BPC] = res.results[c]["out"]
    return out, res


def kernel(query, key, value, valid_length):
    out, _ = _run(dict(query=query, key=key, value=value,
                       valid_length=valid_length))
    return out


# revision 6
# speedup vs baseline: 1.0283x; 1.0283x over previous
"""Masked cross-attention kernel for Trainium2 (8 NeuronCores, SPMD).

Problem: B=16 batches of softmax(mask(Q@K^T/sqrt(D)))@V with
Lq=Lk=2048, D=DV=256.  The reference zeroes masked scores (NOT -inf)
before the softmax, so masked keys still contribute exp(0)=1 to the
denominator and weight 1/denom on V rows.

Strategy (all host prep is exact):
  * Zero K rows at k >= valid_length[b] on the host.  Then Q @ K^T is
    *exactly* 0.0 at masked positions — identical to the reference's
    jnp.where — and no mask tensor is needed on-device.
  * Pre-transpose Q and K to [D, L] layout on the host so both matmul
    operands stream naturally (contraction on the partition dim).
  * Append a ones-column to V.  P @ [V | 1] then yields the softmax
    denominator as output column 256 for free.
  * bf16 matmul inputs (fp32 PSUM accumulate), fp32 softmax math.

Per core: 2 batches.  Per batch, for each 512-wide q tile:
  stage 1: S^T[k,q] tiles in PSUM (KtT.T @ Qt), exp via ScalarE
           (scale=1/16 folded in) -> P^T bf16 in SBUF
  stage 2: O[q,v] = (P^T).T @ [V|1] accumulated over k chunks in PSUM;
           divide by column 256 (DVE reciprocal + per-partition mul).
Stage 1 of q-tile i+1 is emitted before stage 2 of q-tile i so the PE
never stalls on the ScalarE exp chain.
"""

import numpy as np
import ml_dtypes

import concourse.bass as bass
import concourse.mybir as mybir
import concourse.tile as tile
from concourse import bacc
from concourse.bass_utils import run_bass_kernel_spmd

B, LQ, LK, D, DV = 16, 2048, 2048, 256, 256
N_CORES = 8
BPC = B // N_CORES  # batches per core

QT = 512            # q-tile width (stage-1 moving free dim)
NQT = LQ // QT      # 4
KT = 128            # k-tile (partition dim of S^T)
NKT = LK // KT      # 16
KG = 2              # k-tiles per PSUM/exp group
NKG = NKT // KG     # 8
NDC = D // 128      # contraction chunks (2)
QS = 128            # q-subtile for stage 2
NQS = QT // QS      # 4
VF = DV + 1         # 257: V plus the ones column
WARMUP_MMS = 8      # HAM warm-up zero-matmuls before the first real MM

_BF16 = mybir.dt.bfloat16
_F32 = mybir.dt.float32

_NC_CACHE = {}


def _build_nc():
    nc = bacc.Bacc("TRN2", target_bir_lowering=False, debug=False,
                   num_devices=N_CORES)

    qt_d = nc.declare_dram_parameter("qt", [BPC, D, LQ], _BF16, isOutput=False)
    kt_d = nc.declare_dram_parameter("kt", [BPC, D, LK], _BF16, isOutput=False)
    v1_d = nc.declare_dram_parameter("v1", [BPC, LK, VF], _BF16, isOutput=False)
    out_d = nc.declare_dram_parameter("out", [BPC, LQ, DV], _F32, isOutput=True)

    with tile.TileContext(nc) as tc:
        with (
            tc.tile_pool(name="qk", bufs=2) as qk_pool,
            tc.tile_pool(name="v", bufs=2) as v_pool,
            tc.tile_pool(name="p", bufs=2) as p_pool,
            tc.tile_pool(name="osb", bufs=4) as o_pool,
            tc.tile_pool(name="small", bufs=8) as small_pool,
            tc.tile_pool(name="ps_s", bufs=2, space="PSUM") as ps_s,
            tc.tile_pool(name="ps_o", bufs=4, space="PSUM") as ps_o,
        ):
            def load_batch(b):
                # Split loads so the first q-tile's operands land ASAP:
                # kt per d-chunk, qt per q-tile; v1 (stage-2 only) after
                # the first q chunk.
                kt_sb = [qk_pool.tile([128, LK], _BF16, tag=f"kt{c}",
                                      name=f"kt{c}_b{b}")
                         for c in range(NDC)]
                for c in range(NDC):
                    nc.sync.dma_start(
                        out=kt_sb[c], in_=kt_d[b, c * 128:(c + 1) * 128, :])
                qt_view = qt_d[b].rearrange("(c p) q -> p c q", p=128)
                qt_sb = [qk_pool.tile([128, NDC, QT], _BF16, tag=f"qt{qi}",
                                      name=f"qt{qi}_b{b}")
                         for qi in range(NQT)]
                nc.sync.dma_start(out=qt_sb[0], in_=qt_view[:, :, 0:QT])
                v1_sb = v_pool.tile([128, NKT, VF], _BF16, tag="v1")
                nc.sync.dma_start(
                    out=v1_sb, in_=v1_d[b].rearrange("(t p) v -> p t v", p=128))
                for qi in range(1, NQT):
                    nc.sync.dma_start(
                        out=qt_sb[qi],
                        in_=qt_view[:, :, qi * QT:(qi + 1) * QT])
                return kt_sb, qt_sb, v1_sb

            def stage1(state, qi, warm=None):
                """S^T = Kt.T @ Qt for one 512-wide q tile; exp -> P^T bf16."""
                kt_sb, qt_sb, _ = state
                p_sb = p_pool.tile([128, NKT * QT], _BF16, tag="p")
                for g in range(NKG):
                    ps = ps_s.tile([128, KG * QT], _F32, tag="s")
                    for h in range(KG):
                        kj = g * KG + h
                        started = False
                        if g == 0 and h == 0 and warm is not None:
                            # HAM warm-up during the initial DMA wait:
                            # zero-matmuls accumulating 0 into this group.
                            for w in range(WARMUP_MMS):
                                nc.tensor.matmul(
                                    ps[:, h * QT:(h + 1) * QT],
                                    lhsT=warm[:, :128], rhs=warm,
                                    start=(w == 0), stop=False)
                            started = True
                        for c in range(NDC):
                            nc.tensor.matmul(
                                ps[:, h * QT:(h + 1) * QT],
                                lhsT=kt_sb[c][:, kj * KT:(kj + 1) * KT],
                                rhs=qt_sb[qi][:, c, :],
                                start=(c == 0 and not started),
                                stop=(c == NDC - 1),
                            )
                    nc.scalar.activation(
                        out=p_sb[:, g * KG * QT:(g + 1) * KG * QT], in_=ps,
                        func=mybir.ActivationFunctionType.Exp,
                        scale=1.0 / 16.0)
                return p_sb

            def stage2(state, b, qi, p_sb):
                """O = P @ [V|1]; normalize by the ones column; DMA out."""
                _, _, v1_sb = state
                for s in range(NQS):
                    o_ps = ps_o.tile([128, VF], _F32, tag="o")
                    for kj in range(NKT):
                        nc.tensor.matmul(
                            o_ps,
                            lhsT=p_sb[:, kj * QT + s * QS:kj * QT + (s + 1) * QS],
                            rhs=v1_sb[:, kj, :],
                            start=(kj == 0), stop=(kj == NKT - 1),
                        )
                    recip = small_pool.tile([128, 1], _F32, tag="r")
                    nc.vector.reciprocal(out=recip, in_=o_ps[:, DV:DV + 1])
                    o_sb = o_pool.tile([128, DV], _F32, tag="o_sb")
                    nc.vector.tensor_scalar_mul(
                        out=o_sb, in0=o_ps[:, :DV], scalar1=recip)
                    q0 = qi * QT + s * QS
                    nc.sync.dma_start(out=out_d[b, q0:q0 + QS, :], in_=o_sb)

            warm = small_pool.tile([128, QT], _BF16, tag="warm")
            nc.vector.memset(warm, 0.0)

            states = [load_batch(b) for b in range(BPC)]
            work = [(b, qi) for b in range(BPC) for qi in range(NQT)]
            pending = None  # (state, b, qi, p_sb)
            for b, qi in work:
                p_sb = stage1(states[b], qi,
                              warm=warm if (b == 0 and qi == 0) else None)
                if pending is not None:
                    stage2(*pending)
                pending = (states[b], b, qi, p_sb)
            stage2(*pending)

    nc.compile()
    return nc


def _get_nc():
    if "nc" not in _NC_CACHE:
        _NC_CACHE["nc"] = _build_nc()
    return _NC_CACHE["nc"]


def _prepare(query, key, value, valid_length):
    query = np.asarray(query, dtype=np.float32)
    key = np.asarray(key, dtype=np.float32)
    value = np.asarray(value, dtype=np.float32)
    valid_length = np.asarray(valid_length)

    kz = key.copy()
    for b in range(B):
        kz[b, int(valid_length[b]):, :] = 0.0

    bf16 = ml_dtypes.bfloat16
    qt = np.ascontiguousarray(query.transpose(0, 2, 1)).astype(bf16)
    kt = np.ascontiguousarray(kz.transpose(0, 2, 1)).astype(bf16)
    v1 = np.concatenate(
        [value, np.ones((B, LK, 1), np.float32)], axis=-1).astype(bf16)
    return qt, kt, v1


def _run(inputs, trace=False):
    qt, kt, v1 = _prepare(**inputs)
    in_maps = [
        {"qt": qt[c * BPC:(c + 1) * BPC],
         "kt": kt[c * BPC:(c + 1) * BPC],
         "v1": v1[c * BPC:(c + 1) * BPC]}
        for c in range(N_CORES)
    ]
    nc = _get_nc()
    res = run_bass_kernel_spmd(nc, in_maps, core_ids=list(range(N_CORES)),
                               trace=trace)
    out = np.empty((B, LQ, DV), np.float32)
    for c in range(N_CORES):
        out[c * BPC:(c + 1) * BPC] = res.results[c]["out"]
    return out, res


def kernel(query, key, value, valid_length):
    out, _ = _run(dict(query=query, key=key, value=value,
                       valid_length=valid_length))
    return out
